# revision 15
# baseline (speedup 1.0000x reference)
"""DiffMoE MLP (8 experts, capacity 1.0) — expert-parallel across 8 TRN2 NeuronCores.

Contract: kernel(**full_inputs) -> full output (4, 2048, 1024) f32.

Strategy (expert-parallel, sharding_hint):
  host   : gating scores + per-expert top-k (bit-identical jnp ops to the
           reference), token gather + fp32 LayerNorm of the gathered tokens,
           weight re-layout into the PE stationary-block format, and the
           final topk-weight scale + scatter-add combine.
  device : core e owns expert e and computes the local expert GEMMs in
           [d, t] layout: fc1 -> fused bias+gelu (ScalarE) -> fc2 -> bias,
           bf16 matmuls with fp32 PSUM accumulation, N=512 moving tiles,
           each stationary block reused for both token halves (halves the
           effective LDWEIGHTS cost), w2 resident in SBUF, DMA spread over
           both HWDGE queues.
"""

import sys

for _p in ("/opt/trn_rl_repo", "/root/.axon_site/_ro/trn_rl_repo"):
    if _p not in sys.path:
        sys.path.append(_p)

import numpy as np
import ml_dtypes

import concourse.bass as bass
import concourse.bacc as bacc
import concourse.tile as tile
from concourse import mybir
from concourse.bass_utils import run_bass_kernel_spmd

BF16 = ml_dtypes.bfloat16

D = 1024          # embed dim
F = 4096          # hidden dim
N_EXP = 8         # experts == cores
BS = 8192         # tokens
K_TOK = 1024      # tokens kept per expert (BS * capacity / n_exp)
LN_EPS = 1e-5

P = 128
KD = D // P       # 8   d-chunks
KF = F // P       # 32  f-chunks
TH = 512          # moving-operand free dim per matmul (one PSUM bank)
NT = K_TOK // TH  # 2   token halves

_NC_CACHE = {}


def _build_nc(debug=False, reps=1):
    nc = bacc.Bacc("TRN2", target_bir_lowering=False, debug=debug)
    f32 = mybir.dt.float32
    bf16 = mybir.dt.bfloat16

    ynt = nc.dram_tensor("ynt", [D, K_TOK], bf16, kind="ExternalInput")
    w1s = nc.dram_tensor("w1s", [KF, P, KD * P], bf16, kind="ExternalInput")
    w2s = nc.dram_tensor("w2s", [KF, P, KD * P], bf16, kind="ExternalInput")
    b1r = nc.dram_tensor("b1r", [P, KF], f32, kind="ExternalInput")
    b2r = nc.dram_tensor("b2r", [P, KD], f32, kind="ExternalInput")
    ot = nc.dram_tensor("ot", [D, K_TOK], f32, kind="ExternalOutput")

    with tile.TileContext(nc) as tc:
        with (
            tc.tile_pool(name="singles", bufs=1) as singles,
            tc.tile_pool(name="big", bufs=1) as big,
            tc.tile_pool(name="w1p", bufs=4) as w1p,
            tc.tile_pool(name="outp", bufs=4) as outp,
            tc.tile_pool(name="psum", bufs=8, space="PSUM") as psum,
        ):
          for _rep in range(reps):
            # ---- prefetch the first fc1 weight stripe ahead of everything
            # on the SP queue so the first matmul isn't gated on it ----
            w1_pre = w1p.tile([P, KD, P], bf16, name="w1pre")
            nc.sync.dma_start(out=w1_pre, in_=w1s[0])

            # ---- token activations (pre-normalized on host), interleaved
            # across both HWDGE queues so chunk k lands ~0.8*(k/2) us in ----
            yn_sb = big.tile([P, KD * K_TOK], bf16)
            for k in range(KD):
                eng = nc.scalar if k % 2 == 0 else nc.sync
                eng.dma_start(
                    out=yn_sb[:, k * K_TOK:(k + 1) * K_TOK],
                    in_=ynt[k * P:(k + 1) * P, :],
                )

            # ---- small constants on the ACT queue ----
            b1_sb = singles.tile([P, KF], f32)
            nc.scalar.dma_start(out=b1_sb, in_=b1r[:])
            b2_sb = singles.tile([P, KD], f32)
            nc.scalar.dma_start(out=b2_sb, in_=b2r[:])

            # ---- fc1 + gelu: h^T[f, t] = gelu(W1^T.T @ yn + b1) ----
            h_sb = big.tile([P, KF * K_TOK], bf16)
            w2_sb = big.tile([P, KF * KD * P], bf16)
            for m in range(KF):
                if m == 0:
                    w1t = w1_pre
                else:
                    w1t = w1p.tile([P, KD, P], bf16)
                    nc.sync.dma_start(out=w1t, in_=w1s[m])
                pss = [psum.tile([P, TH], mybir.dt.float32, tag="ps",
                                 name=f"ps1_{m}_{t}") for t in range(NT)]
                for k in range(KD):
                    for t in range(NT):
                        # consecutive matmuls share the stationary operand
                        nc.tensor.matmul(
                            pss[t], w1t[:, k, :],
                            yn_sb[:, k * K_TOK + t * TH: k * K_TOK + (t + 1) * TH],
                            start=(k == 0), stop=(k == KD - 1),
                        )
                for t in range(NT):
                    nc.scalar.activation(
                        h_sb[:, m * K_TOK + t * TH: m * K_TOK + (t + 1) * TH],
                        pss[t], mybir.ActivationFunctionType.Gelu_apprx_tanh,
                        bias=b1_sb[:, m:m + 1], scale=1.0,
                    )
                # interleave the resident-w2 loads with fc1 weight streaming
                nc.sync.dma_start(
                    out=w2_sb[:, m * KD * P:(m + 1) * KD * P], in_=w2s[m])

            # ---- fc2 + bias: o^T[d, t] ----
            # rounds of 3/3/2 d-chunks: at most 6 PSUM banks live, so the
            # next round's matmuls never wait on evictions, and the final
            # (tail-exposed) round has only 4 tiles to drain
            for ms in (range(0, 3), range(3, 6), range(6, 8)):
                ps2 = {(m, t): psum.tile([P, TH], mybir.dt.float32, tag="ps",
                                         name=f"ps2_{m}_{t}")
                       for m in ms for t in range(NT)}
                for k in range(KF):
                    for m in ms:
                        w2blk = w2_sb[:, k * KD * P + m * P: k * KD * P + (m + 1) * P]
                        for t in range(NT):
                            # consecutive matmuls share the stationary operand
                            nc.tensor.matmul(
                                ps2[(m, t)], w2blk,
                                h_sb[:, k * K_TOK + t * TH: k * K_TOK + (t + 1) * TH],
                                start=(k == 0), stop=(k == KF - 1))
                for m in ms:
                    for t in range(NT):
                        o_t = outp.tile([P, TH], f32)
                        if (m + t) % 2 == 0:
                            nc.scalar.activation(o_t, ps2[(m, t)],
                                                 mybir.ActivationFunctionType.Identity,
                                                 bias=b2_sb[:, m:m + 1], scale=1.0)
                        else:
                            nc.vector.tensor_scalar_add(o_t, ps2[(m, t)],
                                                        b2_sb[:, m:m + 1])
                        eng = nc.sync if (m + t) % 2 == 0 else nc.scalar
                        eng.dma_start(
                            out=ot[m * P:(m + 1) * P, t * TH:(t + 1) * TH],
                            in_=o_t,
                        )

    nc.compile()
    return nc


def get_nc():
    if "nc" not in _NC_CACHE:
        _NC_CACHE["nc"] = _build_nc()
    return _NC_CACHE["nc"]


def _gate_topk(xf32, gate_w):
    """Replicates the reference gating bit-exactly (same jnp ops, same backend)."""
    import jax
    import jax.numpy as jnp

    xf = jnp.asarray(xf32)
    gw = jnp.asarray(np.asarray(gate_w, dtype=np.float32))
    scores = xf @ gw.T
    scores = (jnp.tanh(scores) + 1.0) * 0.5
    vals, idx = jax.lax.top_k(scores.T, K_TOK)   # (n, k)
    return np.asarray(vals), np.asarray(idx)


def kernel(x, gate_w, ln_gamma, ln_beta, fc1s, b1s, fc2s, b2s):
    x = np.asarray(x, dtype=np.float32)
    gate_w = np.asarray(gate_w, dtype=np.float32)
    ln_gamma = np.asarray(ln_gamma, dtype=np.float32)
    ln_beta = np.asarray(ln_beta, dtype=np.float32)
    fc1s = np.asarray(fc1s, dtype=np.float32)
    b1s = np.asarray(b1s, dtype=np.float32)
    fc2s = np.asarray(fc2s, dtype=np.float32)
    b2s = np.asarray(b2s, dtype=np.float32)

    og_shape = x.shape
    xf = x.reshape(-1, D)
    vals, idx = _gate_topk(xf, gate_w)

    np_inputs = {"ln_gamma": ln_gamma, "ln_beta": ln_beta,
                 "fc1s": fc1s, "b1s": b1s, "fc2s": fc2s, "b2s": b2s}
    in_maps = build_in_maps(np_inputs, xf, vals, idx)

    nc = get_nc()
    res = run_bass_kernel_spmd(nc, in_maps, core_ids=list(range(N_EXP)))

    out = xf.copy()
    for e in range(N_EXP):
        o_e = np.asarray(res.results[e]["ot"]).T           # (k, d) f32
        out[idx[e]] += o_e * vals[e][:, None]
    return out.reshape(og_shape)


def build_in_maps(np_inputs, xf, vals, idx):
    gam = np_inputs["ln_gamma"]
    bet = np_inputs["ln_beta"]
    maps = []
    for e in range(N_EXP):
        y_e = xf[idx[e]]                                   # (k, d) f32
        mu = y_e.mean(axis=1, keepdims=True)
        var = y_e.var(axis=1, keepdims=True)
        yn = (y_e - mu) / np.sqrt(var + LN_EPS) * gam + bet
        maps.append({
            "ynt": np.ascontiguousarray(yn.T).astype(BF16),
            "w1s": np.ascontiguousarray(
                np_inputs["fc1s"][e].reshape(KF, P, KD, P).transpose(0, 3, 2, 1)
            ).reshape(KF, P, KD * P).astype(BF16),
            "w2s": np.ascontiguousarray(
                np_inputs["fc2s"][e].reshape(KD, P, KF, P).transpose(2, 3, 0, 1)
            ).reshape(KF, P, KD * P).astype(BF16),
            "b1r": np.ascontiguousarray(np_inputs["b1s"][e].reshape(KF, P).T),
            "b2r": np.ascontiguousarray(np_inputs["b2s"][e].reshape(KD, P).T),
        })
    return maps
